# revision 10
# baseline (speedup 1.0000x reference)
"""Trainium2 Bass kernel for nn_EnvironmentSpecificDecoder.

Data-parallel over batch B=32 across 8 NeuronCores (NB=4 batches/core).

All matmuls bf16 (1 col/PE-slot, FWL weight loads, half input DMA bytes).
S23 uses zero-padded full-K=128 weights ([W;0] / [0;W] blocks, the same
trick C1 uses) instead of fp32r row-pairing: streams at bf16 rate and
avoids the fp32 cast + fp32r weight tables.

Queue/engine split (each engine issuing DMAs gets its own HW queue, and
DMA_DIRECT2D costs ~600ns of issuing-engine time):
  sync (SP):     zin input stream (b0 in 4 chunks for fast start, b1-3
                 whole-batch), b1-3 dispatched weights, per-oct-pair
                 mu + sigma-staging extraction DMAs.
  scalar (Act):  prologue weight DMAs (reg, wpk, bc, b0 dispatch) so the
                 input stream never queues behind weights. In-loop ACTs.
  gpsimd (Pool): w1p zero-fill memsets, sigma +0.01, per-batch sg output.

Outputs are written in kernel-natural dense layouts (4KB runs) and
unpermuted on the host; sigma softplus runs per batch, overlapped with
the next batch's compute, instead of in a serial epilogue.

Layout: pair pr=(qq,t01) holds (t, t+2) over tp; per-oct intermediates
are t-ascending. Per batch b, oct o (8 t's):
  stage1: 4 MMs  p1[(tp,l),(qq,t01,i)] = zz_pair^T @ A          (N=128)
  C1    : 4 MMs  pc[h2,(t01,i)] = Wc_pad^T zcT                  (N=256)
  S23   : per hh: 2 MMs ph[h,(tp half, qq,t01,i)] = W1p^T zzt   (N=512)
          W1p = [[W1s;0]|[0;W1s]] blocks, W1s = W_sig@W1[env] fused on
          host (env dispatched per batch by regime via dynamic DMA)
  evac  : relu+bias fused PSUM->SBUF bf16 casts split Scalar/Vector
  S4+C2 : per quad 3 accumulating MMs (W2 halves + Wo, zero-padded to
          M=32) col-packed 2 quads per PSUM bank at partition bases 0/32
  out   : one [64,512] bias ACT per oct into a 2-oct staging tile; per
          oct-pair one DMA lifts mu rows {0,32} to DRAM and one lifts
          sigma rows {1,33} into a dense [16,512] block; per batch:
          softplus (exp+ln+add) + one dense sg DMA.
"""
import numpy as np
import ml_dtypes

N_CORES = 8
NB = 4          # batches per core
T = 64
D = 128
L = 64
H = 256
H2 = 128
NE = 8

_CACHE = {}


def _build():
    import concourse.bacc as bacc
    import concourse.bass as bass
    import concourse.mybir as mybir
    from concourse.tile import TileContext

    F32 = mybir.dt.float32
    BF16 = mybir.dt.bfloat16
    AF = mybir.ActivationFunctionType
    ADD = mybir.AluOpType.add
    MAX = mybir.AluOpType.max
    ACT_E = mybir.EngineType.Activation
    SP_E = mybir.EngineType.SP

    nc = bacc.Bacc("TRN2", target_bir_lowering=False, debug=False)

    # inputs (host pre-packed, see _prepare_in_maps)
    zin_d = nc.dram_tensor("zin", [NB, 2, D, T * L], BF16, kind="ExternalInput")
    wpk_d = nc.dram_tensor("wpk", [D, 416], BF16, kind="ExternalInput")
    bc_d = nc.dram_tensor("bc", [H2, 1], F32, kind="ExternalInput")
    reg_d = nc.dram_tensor("reg", [1, NB], mybir.dt.int32, kind="ExternalInput")
    w1h_d = nc.dram_tensor("w1h", [NE, L, H], BF16, kind="ExternalInput")
    w2p_d = nc.dram_tensor("w2p", [NE, D, 2, 32], BF16, kind="ExternalInput")
    b12_d = nc.dram_tensor("b12", [NE, D, 3], F32, kind="ExternalInput")

    mu_d = nc.dram_tensor("mu", [NB, 2, 4096], F32, kind="ExternalOutput")
    sg_d = nc.dram_tensor("sg", [NB, 16, 512], F32, kind="ExternalOutput")

    with TileContext(nc) as tc:
        with (
            tc.tile_pool(name="const", bufs=1) as constp,
            tc.tile_pool(name="zin", bufs=2) as zinp,
            tc.tile_pool(name="ev", bufs=2) as evp,
            tc.tile_pool(name="stg", bufs=4) as stgp,
            tc.tile_pool(name="fin", bufs=1) as finp,
            tc.tile_pool(name="ps1", bufs=1, space="PSUM") as ps1,
            tc.tile_pool(name="ps23", bufs=1, space="PSUM") as ps23,
            tc.tile_pool(name="psc", bufs=2, space="PSUM") as psc,
            tc.tile_pool(name="ps4", bufs=1, space="PSUM") as ps4,
        ):
            # ---- scalar-queue prologue: reg + dispatch + static weights ----
            reg_sb = constp.tile([1, NB], mybir.dt.int32)
            nc.scalar.dma_start(reg_sb[:], reg_d[:])

            evals = []
            for b in range(NB):
                eng = ACT_E if b == 0 else SP_E
                evals.append(nc.values_load(
                    reg_sb[0:1, b : b + 1],
                    engines=[eng],
                    min_val=0, max_val=NE - 1,
                    skip_runtime_bounds_check=True,
                ))

            wpk_sb = constp.tile([D, 416], BF16)  # [ai | wc_pad | wo_pad]
            nc.scalar.dma_start(wpk_sb[:], wpk_d[:])
            bc_sb = constp.tile([H2, 1], F32)
            nc.scalar.dma_start(bc_sb[:], bc_d[:])
            ai_sb = wpk_sb[:, 0:128]
            wc_sb = wpk_sb[:, 128:384]
            wo_sb = wpk_sb[:, 384:416]

            w1p_sb, w2_sb, b12_sb = [None] * NB, [None] * NB, [None] * NB

            def load_batch_weights(b):
                # per-batch dispatched weights (regime -> env); b0 on the
                # scalar queue (needed first), b1-3 on sync between inputs
                e = evals[b]
                de = nc.scalar if b == 0 else nc.sync
                w1p = constp.tile([D, 512], BF16, name=f"w1p{b}", tag=f"w1p{b}")
                nc.gpsimd.memset(w1p[:], 0.0)
                src = w1h_d[bass.ds(e, 1)].rearrange("o p h -> (o p) h")
                de.dma_start(w1p[0:64, 0:256], src)
                de.dma_start(w1p[64:128, 256:512], src)
                w2 = constp.tile([D, 2, 32], BF16, name=f"w2{b}", tag=f"w2{b}")
                de.dma_start(
                    w2[:], w2p_d[bass.ds(e, 1)].rearrange("o p a k -> (o p) a k")
                )
                b12 = constp.tile([D, 3], F32, name=f"b12{b}", tag=f"b12{b}")
                de.dma_start(
                    b12[:], b12_d[bass.ds(e, 1)].rearrange("o p h -> (o p) h")
                )
                w1p_sb[b] = w1p
                w2_sb[b] = w2
                b12_sb[b] = b12

            zint = [None] * NB

            def load_input_chunk(b, ck):
                # 1/4-batch chunks so later-queued staging DMAs never sit
                # behind a monolithic 2MB transfer on the sync queue
                cs = 1024 * ck
                nc.sync.dma_start(
                    zint[b][:, :, cs : cs + 1024],
                    zin_d[b, :, :, cs : cs + 1024]
                    .rearrange("a p c -> p a c"),
                )

            load_batch_weights(0)
            zint[0] = zinp.tile([D, 2, T * L], BF16, name="zt0", tag="zt")
            for ck in range(4):
                load_input_chunk(0, ck)

            st_sig = finp.tile([16, NB * 512], F32)
            ex_sig = finp.tile([16, NB * 512], F32)

            for b in range(NB):
                # prefetch next batch: weights + first input chunk now,
                # remaining chunks interleaved after each staging pair so
                # the sync-queue FIFO keeps staging DMAs flowing
                if b + 1 < NB:
                    load_batch_weights(b + 1)
                    zint[b + 1] = zinp.tile([D, 2, T * L], BF16,
                                            name=f"zt{b+1}", tag="zt")
                    load_input_chunk(b + 1, 0)
                zz = zint[b][:, 0]
                zc = zint[b][:, 1]
                b1s = b12_sb[b][:, 0:2]
                b2b = b12_sb[b][:, 2:3]

                for o in range(8):
                    # ---- stage 1: 4 signal pair matmuls, N=128 ----
                    p1 = ps1.tile([D, 512], F32, tag="p1")
                    for qt in range(4):           # qt = qq*2+t01
                        pr = o * 4 + qt
                        nc.tensor.matmul(
                            p1[:, 128 * qt : 128 * (qt + 1)],
                            zz[:, 128 * pr : 128 * (pr + 1)],
                            ai_sb[:],
                            start=True, stop=True,
                        )
                    # ---- C1 (independent of zzt; hides the cast) ----
                    pcs = []
                    for qq in range(2):
                        pc = psc.tile([D, 512], F32, tag="pc")
                        for par in range(2):
                            nc.tensor.matmul(
                                pc[:, 256 * par : 256 * (par + 1)],
                                wc_sb[:, 128 * par : 128 * (par + 1)],
                                zc[:, 512 * o + 256 * qq :
                                   512 * o + 256 * qq + 256],
                                start=True, stop=True,
                            )
                        pcs.append(pc)

                    # ---- stage-1 evacuation: fp32 PSUM -> bf16 SBUF ----
                    zzt = evp.tile([D, 512], BF16, tag="zzt")
                    nc.vector.tensor_copy(zzt[:], p1[:])

                    # ---- S23 (hh-major) + h1 evac right after each hh ----
                    # h1 cols: hh*1024 + qq*512 + tq*128 + i  (tq = 2*tp+t01)
                    h1 = evp.tile([D, 2048], BF16, tag="h1")
                    h1v = h1[:].rearrange(
                        "p (hh qq par c) -> p hh qq par c", hh=2, qq=2, par=2)
                    for hh in range(2):
                        ph = ps23.tile([D, 1024], F32, tag=f"p23h{hh}")
                        for par in range(2):    # par = tp partition half
                            nc.tensor.matmul(
                                ph[:, 512 * par : 512 * par + 512],
                                w1p_sb[b][:, 256 * par + 128 * hh :
                                          256 * par + 128 * hh + 128],
                                zzt[:, :],
                                start=True, stop=True,
                            )
                        in_ap = ph[:].rearrange(
                            "p (par qq c) -> p par qq c", par=2, qq=2
                        ).transpose([0, 2, 1, 3])
                        out_ap = h1v[:, hh]
                        if hh == 0:
                            nc.scalar.activation(
                                out_ap, in_ap, AF.Relu,
                                bias=b1s[:, 0:1],
                            )
                        else:
                            nc.vector.tensor_scalar(
                                out_ap, in_ap,
                                b1s[:, 1:2], 0.0, ADD, MAX,
                            )

                    # ---- hc evac: relu(pc + bc) -> bf16 ----
                    hcs = evp.tile([D, 1024], BF16, tag="hcs")
                    nc.scalar.activation(
                        hcs[:, 0:512], pcs[0][:], AF.Relu, bias=bc_sb[:, 0:1])
                    nc.scalar.activation(
                        hcs[:, 512:768], pcs[1][:, 0:256], AF.Relu,
                        bias=bc_sb[:, 0:1])
                    nc.vector.tensor_scalar(
                        hcs[:, 768:1024], pcs[1][:, 256:512],
                        bc_sb[:, 0:1], 0.0, ADD, MAX)

                    # ---- S4 + C2: col-packed quads, 2 per PSUM bank ----
                    p4 = ps4.tile([D, 512], F32, tag="p4")
                    for qq in range(2):
                        bp = qq * 32
                        nc.tensor.matmul(
                            p4[bp : bp + 32, :], w2_sb[b][:, 0, :],
                            h1[:, 512 * qq : 512 * qq + 512],
                            start=True, stop=False,
                        )
                        nc.tensor.matmul(
                            p4[bp : bp + 32, :], wo_sb[:],
                            hcs[:, 512 * qq : 512 * qq + 512],
                            start=False, stop=False,
                        )
                        nc.tensor.matmul(
                            p4[bp : bp + 32, :], w2_sb[b][:, 1, :],
                            h1[:, 1024 + 512 * qq : 1024 + 512 * qq + 512],
                            start=False, stop=True,
                        )
                    # one bias pass covers both quads (rows 0,1,32,33)
                    if o % 2 == 0:
                        stb2 = stgp.tile([64, 1024], F32, tag="stb")
                    nc.scalar.activation(
                        stb2[:, 512 * (o % 2) : 512 * (o % 2) + 512],
                        p4[0:64, :], AF.Identity,
                        bias=b2b[0:64, 0:1])
                    if o % 2 == 1:
                        k = o // 2
                        for q in range(2):
                            # mu row {32q} -> dense DRAM (host unpermutes)
                            nc.sync.dma_start(
                                mu_d[b, q, 1024 * k : 1024 * k + 1024]
                                .rearrange("(w c) -> w c", w=2),
                                stb2[32 * q : 32 * q + 1, :],
                            )
                            # sigma row {32q+1} -> dense staging block
                            nc.sync.dma_start(
                                st_sig[8 * q + 2 * k : 8 * q + 2 * k + 2,
                                       512 * b : 512 * b + 512],
                                stb2[32 * q + 1 : 32 * q + 2, :],
                            )
                        if b + 1 < NB and k < 3:
                            load_input_chunk(b + 1, k + 1)

                # ---- per-batch sigma: softplus + 0.01 + dense DMA out ----
                sl = slice(512 * b, 512 * b + 512)
                nc.scalar.activation(ex_sig[:, sl], st_sig[:, sl], AF.Exp)
                nc.scalar.activation(st_sig[:, sl], ex_sig[:, sl], AF.Ln,
                                     bias=1.0)
                nc.gpsimd.tensor_scalar_add(st_sig[:, sl], st_sig[:, sl], 0.01)
                # NB: gpsimd SW-DGE DMAs mis-target DRAM on SPMD cores >0;
                # outputs must go through a HWDGE queue (sync/scalar)
                nc.sync.dma_start(sg_d[b], st_sig[:, sl])

    nc.compile()
    return nc


def _get_nc():
    if "nc" not in _CACHE:
        _CACHE["nc"] = _build()
    return _CACHE["nc"]


def _prepare_in_maps(z_signal, z_corrupt, A, regime, W_sig, b_sig, W1e, b1e,
                     W2e, b2e, Wc, bc, Wo, bo):
    bf16 = ml_dtypes.bfloat16
    z_signal = np.asarray(z_signal, dtype=np.float32)
    z_corrupt = np.asarray(z_corrupt, dtype=np.float32)
    A = np.asarray(A, dtype=np.float32)
    regime = np.asarray(regime)
    W_sig = np.asarray(W_sig, dtype=np.float32)
    b_sig = np.asarray(b_sig, dtype=np.float32)
    W1e = np.asarray(W1e, dtype=np.float32)
    b1e = np.asarray(b1e, dtype=np.float32)
    W2e = np.asarray(W2e, dtype=np.float32)
    b2e = np.asarray(b2e, dtype=np.float32)
    Wc = np.asarray(Wc, dtype=np.float32)
    bc = np.asarray(bc, dtype=np.float32)
    Wo = np.asarray(Wo, dtype=np.float32)
    bo = np.asarray(bo, dtype=np.float32)

    eidx = np.where(regime >= NE, 0, regime).astype(np.int32)

    # ---- host weight transforms (env tables, replicated to all cores) ----
    wpk = np.zeros((D, 416), np.float32)
    wpk[:, 0:128] = A
    wpk[0:64, 128:256] = Wc            # [[Wc;0] | [0;Wc]]
    wpk[64:128, 256:384] = Wc
    wpk[:, 384:385] = Wo
    wpk = wpk.astype(bf16)
    w1h = np.einsum("lh,ehk->elk", W_sig, W1e).astype(bf16)    # [E, L, H]
    b1s_full = np.einsum("h,ehk->ek", b_sig, W1e) + b1e        # [E, H]
    b12 = np.zeros((NE, D, 3), np.float32)
    b12[..., 0:2] = b1s_full.reshape(NE, 2, D).transpose(0, 2, 1)
    b12[:, 0::32, 2] = (b2e[:, 0] + bo[0])[:, None]
    b12[:, 1::32, 2] = b2e[:, 1][:, None]
    w2p = np.zeros((NE, D, 2, 32), np.float32)
    w2p[..., 0:2] = W2e.reshape(NE, 2, D, 2).transpose(0, 2, 1, 3)
    w2p = w2p.astype(bf16)
    bc_r = np.ascontiguousarray(bc[:, None])                   # [H2, 1]

    in_maps = []
    for c in range(N_CORES):
        b0 = c * NB
        zs4 = z_signal[b0 : b0 + NB]
        zc4 = z_corrupt[b0 : b0 + NB]
        # signal: [nb, D, (o,qq,t01,tp,l)] — pair pr=(o,qq,t01) holds (t,t+2)
        zt = zs4.transpose(0, 2, 1, 3).reshape(NB, D, 8, 2, 2, 2, L)
        zs_p = zt.transpose(0, 1, 2, 3, 5, 4, 6).reshape(NB, D, T * L)
        # corrupt (host-transposed): [nb, (tp,l), (o,qq,t01,i)]
        zcr = zc4.reshape(NB, 8, 2, 2, 2, D, L)
        zc_p = zcr.transpose(0, 3, 6, 1, 2, 4, 5).reshape(NB, D, T * L)
        zin = np.ascontiguousarray(
            np.stack([zs_p, zc_p], axis=1)).astype(bf16)       # [NB,2,D,TL]
        in_maps.append({
            "zin": zin,
            "wpk": wpk,
            "bc": bc_r,
            "reg": eidx[None, b0 : b0 + NB],
            "w1h": w1h,
            "w2p": w2p,
            "b12": b12,
        })
    return in_maps


def _unpermute(res):
    # mu_d[b, q, o*512 + tq*128 + d] ; sg_d[b, q*8+o, tq*128 + d]
    # t = o*8 + q*4 + tq
    mu = np.concatenate(
        [r["mu"].reshape(NB, 2, 8, 4, D).transpose(0, 2, 1, 3, 4)
         .reshape(NB, T, D) for r in res.results], axis=0)
    sg = np.concatenate(
        [r["sg"].reshape(NB, 2, 8, 4, D).transpose(0, 2, 1, 3, 4)
         .reshape(NB, T, D) for r in res.results], axis=0)
    return mu, sg


def kernel(z_signal, z_corrupt, A, regime, W_sig, b_sig, W1e, b1e, W2e, b2e,
           Wc, bc, Wo, bo):
    from concourse.bass_utils import run_bass_kernel_spmd

    in_maps = _prepare_in_maps(z_signal, z_corrupt, A, regime, W_sig, b_sig,
                               W1e, b1e, W2e, b2e, Wc, bc, Wo, bo)
    nc = _get_nc()
    res = run_bass_kernel_spmd(nc, in_maps, core_ids=list(range(N_CORES)))
    return _unpermute(res)


def run_traced(inputs_np):
    from concourse.bass_utils import run_bass_kernel_spmd

    in_maps = _prepare_in_maps(**inputs_np)
    nc = _get_nc()
    return run_bass_kernel_spmd(
        nc, in_maps, core_ids=list(range(N_CORES)), trace=True
    )


# revision 12
# speedup vs baseline: 1.3798x; 1.3798x over previous
"""Trainium2 Bass kernel for nn_EnvironmentSpecificDecoder.

Data-parallel over batch B=32 across 8 NeuronCores (NB=4 batches/core).

Matmuls in bf16 (FWL weight loads, half the input DMA bytes) except S23
which stays fp32r: its row-paired K=64 tiles stream two concurrent
half-partition matmuls (~0.25 ns/token-col), which beats serial bf16
full-K matmuls (~0.37), and bf16 row-paired hangs the HW.

Queue/engine split (each engine issuing DMAs gets its own HW queue, and
DMA_DIRECT2D costs ~600ns of issuing-engine time):
  sync (SP):     zin input stream (b0 in 2 chunks for fast start, b1-3
                 whole-batch prefetched at prior-batch start), b1-3
                 dispatched weights, per-oct-pair mu/sigma extraction
                 DMAs, final sg writes.
  scalar (Act):  prologue weight DMAs (reg, wpk, bc, b0 dispatch) so the
                 input stream never queues behind weights. In-loop ACTs.

Outputs are written in kernel-natural dense layouts (4KB runs) and
unpermuted on the host; softplus runs once at the end (exp and ln live
in different ACT tables, so interleaving them with the in-loop
relu/identity ACTs would thrash table loads).

Layout: pair pr=(qq,t01) holds (t, t+2) over tp; per-oct intermediates
are t-ascending. Per batch b, oct o (8 t's):
  stage1: 4 MMs  p1[(tp,l),(qq,t01,i)] = zz_pair^T @ A          (N=128)
  C1    : 4 MMs  pc[h2,(t01,i)] = Wc_pad^T zcT                  (N=256)
  S23   : per hh: 2 row-paired concurrent K=64 MMs (fp32r)    (N=512)
          ph[h,(tp,qq,t01,i)] = W1s^T zzt, W1s = W_sig@W1[env] fused on
          host (env dispatched per batch by regime via dynamic DMA)
  evac  : relu+bias fused PSUM->SBUF bf16 casts split Scalar/Vector
  S4+C2 : per quad 3 accumulating MMs (W2 halves + Wo, zero-padded to
          M=32) col-packed 2 quads per PSUM bank at partition bases 0/32
  out   : one [64,512] bias ACT per oct into a 2-oct staging tile; per
          oct-pair one DMA lifts mu rows {0,32} to DRAM and one lifts
          sigma rows {1,33} into a dense [16,512] block; per batch:
          softplus (exp+ln+add) + one dense sg DMA.
"""
import numpy as np
import ml_dtypes

N_CORES = 8
NB = 4          # batches per core
T = 64
D = 128
L = 64
H = 256
H2 = 128
NE = 8

_CACHE = {}


def _round_fp32r(x: np.ndarray) -> np.ndarray:
    """Round fp32 array to E8M11 (float32r) with round-to-nearest-even."""
    u = np.ascontiguousarray(x, dtype=np.float32).view(np.uint32)
    keep = np.uint32(12)
    half = np.uint32(1 << 11)
    lsb = (u >> keep) & np.uint32(1)
    return ((u + (half - np.uint32(1) + lsb)) >> keep << keep).view(np.float32)


def _build():
    import concourse.bacc as bacc
    import concourse.bass as bass
    import concourse.mybir as mybir
    from concourse.tile import TileContext

    F32 = mybir.dt.float32
    F32R = mybir.dt.float32r
    BF16 = mybir.dt.bfloat16
    AF = mybir.ActivationFunctionType
    ADD = mybir.AluOpType.add
    MAX = mybir.AluOpType.max
    ACT_E = mybir.EngineType.Activation
    SP_E = mybir.EngineType.SP

    nc = bacc.Bacc("TRN2", target_bir_lowering=False, debug=False)

    # inputs (host pre-packed, see _prepare_in_maps)
    zin_d = nc.dram_tensor("zin", [NB, 2, D, T * L], BF16, kind="ExternalInput")
    wpk_d = nc.dram_tensor("wpk", [D, 416], BF16, kind="ExternalInput")
    bc_d = nc.dram_tensor("bc", [H2, 1], F32, kind="ExternalInput")
    reg_d = nc.dram_tensor("reg", [1, NB], mybir.dt.int32, kind="ExternalInput")
    w1s_d = nc.dram_tensor("w1s", [NE, D, H], F32R, kind="ExternalInput")
    w2p_d = nc.dram_tensor("w2p", [NE, D, 2, 32], BF16, kind="ExternalInput")
    b12_d = nc.dram_tensor("b12", [NE, D, 3], F32, kind="ExternalInput")

    mu_d = nc.dram_tensor("mu", [NB, 2, 4096], F32, kind="ExternalOutput")
    sg_d = nc.dram_tensor("sg", [NB, 16, 512], F32, kind="ExternalOutput")

    with TileContext(nc) as tc:
        with (
            tc.tile_pool(name="const", bufs=1) as constp,
            tc.tile_pool(name="zin", bufs=2) as zinp,
            tc.tile_pool(name="ev", bufs=2) as evp,
            tc.tile_pool(name="stg", bufs=4) as stgp,
            tc.tile_pool(name="fin", bufs=1) as finp,
            tc.tile_pool(name="ps1", bufs=1, space="PSUM") as ps1,
            tc.tile_pool(name="ps23", bufs=1, space="PSUM") as ps23,
            tc.tile_pool(name="psc", bufs=2, space="PSUM") as psc,
            tc.tile_pool(name="ps4", bufs=1, space="PSUM") as ps4,
        ):
            # ---- scalar-queue prologue: reg + dispatch + static weights ----
            reg_sb = constp.tile([1, NB], mybir.dt.int32)
            nc.scalar.dma_start(reg_sb[:], reg_d[:])

            evals = []
            for b in range(NB):
                eng = ACT_E if b == 0 else SP_E
                evals.append(nc.values_load(
                    reg_sb[0:1, b : b + 1],
                    engines=[eng],
                    min_val=0, max_val=NE - 1,
                    skip_runtime_bounds_check=True,
                ))

            wpk_sb = constp.tile([D, 416], BF16)  # [ai | wc_pad | wo_pad]
            nc.scalar.dma_start(wpk_sb[:], wpk_d[:])
            bc_sb = constp.tile([H2, 1], F32)
            nc.scalar.dma_start(bc_sb[:], bc_d[:])
            ai_sb = wpk_sb[:, 0:128]
            wc_sb = wpk_sb[:, 128:384]
            wo_sb = wpk_sb[:, 384:416]

            w1p_sb, w2_sb, b12_sb = [None] * NB, [None] * NB, [None] * NB

            def load_batch_weights(b):
                # per-batch dispatched weights (regime -> env); b0 on the
                # scalar queue (needed first), b1-3 on sync between inputs
                e = evals[b]
                de = nc.scalar if b == 0 else nc.sync
                w1p = constp.tile([D, H], F32R, name=f"w1p{b}", tag=f"w1p{b}")
                de.dma_start(
                    w1p[:], w1s_d[bass.ds(e, 1)].rearrange("o p h -> (o p) h"))
                w2 = constp.tile([D, 2, 32], BF16, name=f"w2{b}", tag=f"w2{b}")
                de.dma_start(
                    w2[:], w2p_d[bass.ds(e, 1)].rearrange("o p a k -> (o p) a k")
                )
                b12 = constp.tile([D, 3], F32, name=f"b12{b}", tag=f"b12{b}")
                de.dma_start(
                    b12[:], b12_d[bass.ds(e, 1)].rearrange("o p h -> (o p) h")
                )
                w1p_sb[b] = w1p
                w2_sb[b] = w2
                b12_sb[b] = b12

            zint = [None] * NB

            def load_batch_inputs(b, nchunks):
                cw = (T * L) // nchunks
                for ck in range(nchunks):
                    cs = cw * ck
                    nc.sync.dma_start(
                        zint[b][:, :, cs : cs + cw],
                        zin_d[b, :, :, cs : cs + cw]
                        .rearrange("a p c -> p a c"),
                    )

            load_batch_weights(0)
            zint[0] = zinp.tile([D, 2, T * L], BF16, name="zt0", tag="zt")
            load_batch_inputs(0, 2)

            st_sig = finp.tile([16, NB * 512], F32)
            ex_sig = finp.tile([16, NB * 512], F32)

            for b in range(NB):
                # prefetch next batch (weights then inputs) ahead of this
                # batch's staging DMAs on the sync queue; staging waits on
                # compute anyway, so the input stream keeps the lead
                if b + 1 < NB:
                    load_batch_weights(b + 1)
                    zint[b + 1] = zinp.tile([D, 2, T * L], BF16,
                                            name=f"zt{b+1}", tag="zt")
                    load_batch_inputs(b + 1, 1)
                zz = zint[b][:, 0]
                zc = zint[b][:, 1]
                b1s = b12_sb[b][:, 0:2]
                b2b = b12_sb[b][:, 2:3]

                for o in range(8):
                    # ---- stage 1: 4 signal pair matmuls, N=128 ----
                    p1 = ps1.tile([D, 512], F32, tag="p1")
                    for qt in range(4):           # qt = qq*2+t01
                        pr = o * 4 + qt
                        nc.tensor.matmul(
                            p1[:, 128 * qt : 128 * (qt + 1)],
                            zz[:, 128 * pr : 128 * (pr + 1)],
                            ai_sb[:],
                            start=True, stop=True,
                        )
                    # ---- C1 (independent of zzt; hides the cast) ----
                    pcs = []
                    for qq in range(2):
                        pc = psc.tile([D, 512], F32, tag="pc")
                        for par in range(2):
                            nc.tensor.matmul(
                                pc[:, 256 * par : 256 * (par + 1)],
                                wc_sb[:, 128 * par : 128 * (par + 1)],
                                zc[:, 512 * o + 256 * qq :
                                   512 * o + 256 * qq + 256],
                                start=True, stop=True,
                            )
                        pcs.append(pc)

                    # ---- stage-1 evacuation: fp32 PSUM -> bf16 SBUF ----
                    zzt = evp.tile([D, 512], F32R, tag="zzt")
                    nc.vector.tensor_copy(zzt[:], p1[:])

                    # ---- S23 (hh-major) + h1 evac right after each hh ----
                    # h1 cols: hh*1024 + qq*512 + tq*128 + i  (tq = 2*tp+t01)
                    h1 = evp.tile([D, 2048], BF16, tag="h1")
                    h1v = h1[:].rearrange(
                        "p (hh qq par c) -> p hh qq par c", hh=2, qq=2, par=2)
                    for hh in range(2):
                        ph = ps23.tile([D, 1024], F32, tag=f"p23h{hh}")
                        for par in range(2):    # par = tp partition half
                            nc.tensor.matmul(
                                ph[:, 512 * par : 512 * par + 512],
                                w1p_sb[b][64 * par : 64 * par + 64,
                                          128 * hh : 128 * (hh + 1)],
                                zzt[64 * par : 64 * par + 64, :],
                                start=True, stop=True,
                            )
                        in_ap = ph[:].rearrange(
                            "p (par qq c) -> p par qq c", par=2, qq=2
                        ).transpose([0, 2, 1, 3])
                        out_ap = h1v[:, hh]
                        if hh == 0:
                            nc.scalar.activation(
                                out_ap, in_ap, AF.Relu,
                                bias=b1s[:, 0:1],
                            )
                        else:
                            nc.vector.tensor_scalar(
                                out_ap, in_ap,
                                b1s[:, 1:2], 0.0, ADD, MAX,
                            )

                    # ---- hc evac: relu(pc + bc) -> bf16 ----
                    hcs = evp.tile([D, 1024], BF16, tag="hcs")
                    nc.scalar.activation(
                        hcs[:, 0:512], pcs[0][:], AF.Relu, bias=bc_sb[:, 0:1])
                    nc.scalar.activation(
                        hcs[:, 512:768], pcs[1][:, 0:256], AF.Relu,
                        bias=bc_sb[:, 0:1])
                    nc.vector.tensor_scalar(
                        hcs[:, 768:1024], pcs[1][:, 256:512],
                        bc_sb[:, 0:1], 0.0, ADD, MAX)

                    # ---- S4 + C2: col-packed quads, 2 per PSUM bank ----
                    p4 = ps4.tile([D, 512], F32, tag="p4")
                    for qq in range(2):
                        bp = qq * 32
                        nc.tensor.matmul(
                            p4[bp : bp + 32, :], w2_sb[b][:, 0, :],
                            h1[:, 512 * qq : 512 * qq + 512],
                            start=True, stop=False,
                        )
                        nc.tensor.matmul(
                            p4[bp : bp + 32, :], wo_sb[:],
                            hcs[:, 512 * qq : 512 * qq + 512],
                            start=False, stop=False,
                        )
                        nc.tensor.matmul(
                            p4[bp : bp + 32, :], w2_sb[b][:, 1, :],
                            h1[:, 1024 + 512 * qq : 1024 + 512 * qq + 512],
                            start=False, stop=True,
                        )
                    # one bias pass covers both quads (rows 0,1,32,33)
                    if o % 2 == 0:
                        stb2 = stgp.tile([64, 1024], F32, tag="stb")
                    nc.scalar.activation(
                        stb2[:, 512 * (o % 2) : 512 * (o % 2) + 512],
                        p4[0:64, :], AF.Identity,
                        bias=b2b[0:64, 0:1])
                    if o % 2 == 1:
                        k = o // 2
                        for q in range(2):
                            # mu row {32q} -> dense DRAM (host unpermutes)
                            nc.sync.dma_start(
                                mu_d[b, q, 1024 * k : 1024 * k + 1024]
                                .rearrange("(w c) -> w c", w=2),
                                stb2[32 * q : 32 * q + 1, :],
                            )
                            # sigma row {32q+1} -> dense staging block
                            nc.sync.dma_start(
                                st_sig[8 * q + 2 * k : 8 * q + 2 * k + 2,
                                       512 * b : 512 * b + 512],
                                stb2[32 * q + 1 : 32 * q + 2, :],
                            )

            # ---- sigma: softplus + 0.01 (dense, once; avoids per-batch
            # exp/ln ACT-table thrash) ----
            nc.scalar.activation(ex_sig[:], st_sig[:], AF.Exp)
            nc.scalar.activation(st_sig[:], ex_sig[:], AF.Ln, bias=1.0)
            nc.vector.tensor_scalar_add(st_sig[:], st_sig[:], 0.01)
            for b in range(NB):
                nc.sync.dma_start(sg_d[b], st_sig[:, 512 * b : 512 * b + 512])

    nc.compile()
    return nc


def _get_nc():
    if "nc" not in _CACHE:
        _CACHE["nc"] = _build()
    return _CACHE["nc"]


def _prepare_in_maps(z_signal, z_corrupt, A, regime, W_sig, b_sig, W1e, b1e,
                     W2e, b2e, Wc, bc, Wo, bo):
    bf16 = ml_dtypes.bfloat16
    z_signal = np.asarray(z_signal, dtype=np.float32)
    z_corrupt = np.asarray(z_corrupt, dtype=np.float32)
    A = np.asarray(A, dtype=np.float32)
    regime = np.asarray(regime)
    W_sig = np.asarray(W_sig, dtype=np.float32)
    b_sig = np.asarray(b_sig, dtype=np.float32)
    W1e = np.asarray(W1e, dtype=np.float32)
    b1e = np.asarray(b1e, dtype=np.float32)
    W2e = np.asarray(W2e, dtype=np.float32)
    b2e = np.asarray(b2e, dtype=np.float32)
    Wc = np.asarray(Wc, dtype=np.float32)
    bc = np.asarray(bc, dtype=np.float32)
    Wo = np.asarray(Wo, dtype=np.float32)
    bo = np.asarray(bo, dtype=np.float32)

    eidx = np.where(regime >= NE, 0, regime).astype(np.int32)

    # ---- host weight transforms (env tables, replicated to all cores) ----
    wpk = np.zeros((D, 416), np.float32)
    wpk[:, 0:128] = A
    wpk[0:64, 128:256] = Wc            # [[Wc;0] | [0;Wc]]
    wpk[64:128, 256:384] = Wc
    wpk[:, 384:385] = Wo
    wpk = wpk.astype(bf16)
    w1s_half = np.einsum("lh,ehk->elk", W_sig, W1e)            # [E, L, H]
    w1s = _round_fp32r(
        np.ascontiguousarray(np.concatenate([w1s_half, w1s_half], axis=1)))
    b1s_full = np.einsum("h,ehk->ek", b_sig, W1e) + b1e        # [E, H]
    b12 = np.zeros((NE, D, 3), np.float32)
    b12[..., 0:2] = b1s_full.reshape(NE, 2, D).transpose(0, 2, 1)
    b12[:, 0::32, 2] = (b2e[:, 0] + bo[0])[:, None]
    b12[:, 1::32, 2] = b2e[:, 1][:, None]
    w2p = np.zeros((NE, D, 2, 32), np.float32)
    w2p[..., 0:2] = W2e.reshape(NE, 2, D, 2).transpose(0, 2, 1, 3)
    w2p = w2p.astype(bf16)
    bc_r = np.ascontiguousarray(bc[:, None])                   # [H2, 1]

    in_maps = []
    for c in range(N_CORES):
        b0 = c * NB
        zs4 = z_signal[b0 : b0 + NB]
        zc4 = z_corrupt[b0 : b0 + NB]
        # signal: [nb, D, (o,qq,t01,tp,l)] — pair pr=(o,qq,t01) holds (t,t+2)
        zt = zs4.transpose(0, 2, 1, 3).reshape(NB, D, 8, 2, 2, 2, L)
        zs_p = zt.transpose(0, 1, 2, 3, 5, 4, 6).reshape(NB, D, T * L)
        # corrupt (host-transposed): [nb, (tp,l), (o,qq,t01,i)]
        zcr = zc4.reshape(NB, 8, 2, 2, 2, D, L)
        zc_p = zcr.transpose(0, 3, 6, 1, 2, 4, 5).reshape(NB, D, T * L)
        zin = np.ascontiguousarray(
            np.stack([zs_p, zc_p], axis=1)).astype(bf16)       # [NB,2,D,TL]
        in_maps.append({
            "zin": zin,
            "wpk": wpk,
            "bc": bc_r,
            "reg": eidx[None, b0 : b0 + NB],
            "w1s": w1s,
            "w2p": w2p,
            "b12": b12,
        })
    return in_maps


def _unpermute(res):
    # mu_d[b, q, o*512 + tq*128 + d] ; sg_d[b, q*8+o, tq*128 + d]
    # t = o*8 + q*4 + tq
    mu = np.concatenate(
        [r["mu"].reshape(NB, 2, 8, 4, D).transpose(0, 2, 1, 3, 4)
         .reshape(NB, T, D) for r in res.results], axis=0)
    sg = np.concatenate(
        [r["sg"].reshape(NB, 2, 8, 4, D).transpose(0, 2, 1, 3, 4)
         .reshape(NB, T, D) for r in res.results], axis=0)
    return mu, sg


def kernel(z_signal, z_corrupt, A, regime, W_sig, b_sig, W1e, b1e, W2e, b2e,
           Wc, bc, Wo, bo):
    from concourse.bass_utils import run_bass_kernel_spmd

    in_maps = _prepare_in_maps(z_signal, z_corrupt, A, regime, W_sig, b_sig,
                               W1e, b1e, W2e, b2e, Wc, bc, Wo, bo)
    nc = _get_nc()
    res = run_bass_kernel_spmd(nc, in_maps, core_ids=list(range(N_CORES)))
    return _unpermute(res)


def run_traced(inputs_np):
    from concourse.bass_utils import run_bass_kernel_spmd

    in_maps = _prepare_in_maps(**inputs_np)
    nc = _get_nc()
    return run_bass_kernel_spmd(
        nc, in_maps, core_ids=list(range(N_CORES)), trace=True
    )
